# revision 10
# baseline (speedup 1.0000x reference)
"""Causal self-attention (B=2, T=2048, D=1024, H=16) on 8 TRN2 NeuronCores.

Sharding: data parallel on batch (2 groups of 4 cores) x tensor parallel on
heads (4 heads per core, splitting the qkv / out projections). Each core
computes a partial output ``X[b] -> partial_out`` for its 4 heads; the host
sums the 4 partials per batch. No device collectives.

Per-core math (all matmuls fp32r at full PE rate, N>=256):
  qT/kT  [e,t] = Wqkv_localT.T @ X[b].T          (q pre-scaled by 1/8 on host)
  v      [t,d] = X[b] @ Wv_localT                (natural layout)
  scoresT[k,q] = kT.T @ qT                       (contraction d=64)
  attT   [k,q] = exp(scoresT)   * causal mask    (no max subtraction: |scores|
                                                  is O(1) for this data)
  zT     [d,q] = v_ext.T @ attT                  (v_ext has a ones column ->
                                                  row 64 of zT = softmax denom)
  out    [t,e] = (zT/denom).T @ Wout_localT      (host sums partials over 4
                                                  cores per batch)

Pipelining: the projection work for q-block qi (qk-proj of t-block qi,
v-proj of the k-tiles qi unlocks) is emitted just before attention of qi,
so the PE fills the ACT-bound exp phases with projection matmuls. Heads are
processed in interleaved pairs to deepen the sc->exp->mask->av pipeline.
"""

import sys

sys.path.insert(0, "/opt/trn_rl_repo")

import numpy as np

import concourse.bacc as bacc
import concourse.mybir as mybir
import concourse.tile as tile

B, T, D, H = 2, 2048, 1024, 16
HD = D // H              # 64
NCORES = 8
NGROUP = 4               # cores per batch (tensor-parallel group)
HPC = H // NGROUP        # heads per core = 4
DLOC = HPC * HD          # local model dims per core = 256
QB = 512                 # q-block (matmul moving free dim)
NQB = T // QB            # 4
KT = 128                 # k-tile (psum partitions)
NKT = T // KT            # 16
NDT = D // 128           # 8 d-tiles

F32 = mybir.dt.float32
F32R = mybir.dt.float32r


def build_program(reps=1, debug_taps=False):
    nc = bacc.Bacc("TRN2", target_bir_lowering=False, debug=False,
                   num_devices=NCORES)

    xt_d = nc.declare_dram_parameter("xt", [D, T], F32R, isOutput=False)
    wqkv_d = nc.declare_dram_parameter("wqkv", [D, 3 * DLOC], F32R, isOutput=False)
    wout_d = nc.declare_dram_parameter("wout", [DLOC, D], F32R, isOutput=False)
    mask_d = nc.declare_dram_parameter("masks", [KT, 2, 2, QB], F32, isOutput=False)
    ones_d = nc.declare_dram_parameter("ones", [128, NKT, HPC, 1], F32R, isOutput=False)
    out_d = nc.declare_dram_parameter("out", [T, D], F32, isOutput=True)
    if debug_taps:
        dbg = {
            "dbg_qT": nc.declare_dram_parameter("dbg_qT", [128, 2, T], F32R, isOutput=True),
            "dbg_kT": nc.declare_dram_parameter("dbg_kT", [128, 2, T], F32R, isOutput=True),
            "dbg_vext": nc.declare_dram_parameter("dbg_vext", [128, NKT, HPC, HD + 1], F32R, isOutput=True),
            "dbg_at0": nc.declare_dram_parameter("dbg_at0", [128, 2, QB], F32R, isOutput=True),
            "dbg_zt": nc.declare_dram_parameter("dbg_zt", [128, QB], F32, isOutput=True),
            "dbg_scr": nc.declare_dram_parameter("dbg_scr", [128, QB], F32, isOutput=True),
            "dbg_bc": nc.declare_dram_parameter("dbg_bc", [128, QB], F32, isOutput=True),
            "dbg_zTn": nc.declare_dram_parameter("dbg_zTn", [128, 2, QB], F32R, isOutput=True),
        }

    with tile.TileContext(nc) as tc:
        with (
            tc.tile_pool(name="cst", bufs=1) as cst,
            tc.tile_pool(name="att", bufs=4) as attp,
            tc.tile_pool(name="sm", bufs=2) as smp,
            tc.tile_pool(name="ops", bufs=2) as opsp,
            tc.tile_pool(name="ps", bufs=3, space="PSUM") as ps,    # [128,2,512]
            tc.tile_pool(name="zps", bufs=2, space="PSUM") as zps,  # [128,512]
        ):
            xt = cst.tile([128, NDT, T], F32R, tag="xt")
            wqkv = cst.tile([128, NDT, 3 * DLOC], F32R, tag="wqkv")
            wout = cst.tile([128, DLOC // 128, D], F32R, tag="wout")
            masks = cst.tile([128, 2, 2, QB], F32, tag="masks")
            qT = cst.tile([128, 2, T], F32R, tag="qT")
            kT = cst.tile([128, 2, T], F32R, tag="kT")
            vext = cst.tile([128, NKT, HPC, HD + 1], F32R, tag="vext")

            def qk_proj(tb):
                for pair in range(2):  # 0 -> q (e 0:256), 1 -> k
                    pt = ps.tile([128, 2, QB], F32, tag="ps")
                    for j in range(2):
                        ecol = (pair * 2 + j) * 128
                        for dt_ in range(NDT):
                            nc.tensor.matmul(
                                pt[:, j, :],
                                wqkv[:, dt_, ecol:ecol + 128],
                                xt[:, dt_, tb * QB:(tb + 1) * QB],
                                start=(dt_ == 0), stop=(dt_ == NDT - 1),
                            )
                    dst = qT if pair == 0 else kT
                    nc.scalar.copy(dst[:, :, tb * QB:(tb + 1) * QB], pt[:])

            def v_proj(tp):
                pt = ps.tile([128, 2, QB], F32, tag="ps")
                for j in range(2):
                    tch = tp * 2 + j
                    for dt_ in range(NDT):
                        nc.tensor.matmul(
                            pt[:, j, 0:DLOC],
                            xt[:, dt_, tch * 128:(tch + 1) * 128],
                            wqkv[:, dt_, 2 * DLOC:3 * DLOC],
                            start=(dt_ == 0), stop=(dt_ == NDT - 1),
                        )
                    nc.vector.tensor_copy(
                        vext[:, tch, :, 0:HD],
                        pt[:, j, 0:DLOC].rearrange("p (h d) -> p h d", h=HPC),
                    )

            def body(_i):
                # split big loads across DMA queues; xt t-chunks arrive in
                # the order the projections consume them
                for dt_ in range(0, NDT, 2):
                    nc.sync.dma_start(
                        wqkv[:, dt_:dt_ + 2, :],
                        wqkv_d.rearrange("(a p) e -> p a e", p=128)[:, dt_:dt_ + 2, :])
                for tchunk in range(4):
                    sl = slice(tchunk * QB, (tchunk + 1) * QB)
                    nc.sync.dma_start(
                        xt[:, :, sl],
                        xt_d.rearrange("(a p) t -> p a t", p=128)[:, :, sl])
                nc.sync.dma_start(wout[:], wout_d.rearrange("(a p) e -> p a e", p=128))
                nc.sync.dma_start(masks[:], mask_d[:])
                nc.sync.dma_start(vext[:, :, :, HD:HD + 1], ones_d[:])

                for qi in range(NQB):
                    with nc.named_scope(f"proj_q{qi}"):
                        qk_proj(qi)
                        v_proj(2 * qi)
                        v_proj(2 * qi + 1)
                    if debug_taps and qi == NQB - 1:
                        nc.sync.dma_start(dbg["dbg_qT"][:], qT[:])
                        nc.sync.dma_start(dbg["dbg_kT"][:], kT[:])
                        nc.sync.dma_start(dbg["dbg_vext"][:], vext[:])

                    zTn = smp.tile([128, 2, QB], F32R, tag="zTn")
                    G = 2 * (qi + 1)          # k-groups of 2 k-tiles
                    for p in range(2):        # head pairs (0,1) then (2,3)
                        zts = {}
                        att_tiles = {}
                        with nc.named_scope(f"att_q{qi}_p{p}"):
                            for g in range(G + 1):
                                for h in (2 * p, 2 * p + 1):
                                    off, hv = (h % 2) * 64, h // 2
                                    if g < G:
                                        if g == 0:
                                            zts[h] = zps.tile([128, QB], F32, tag="zt", name=f"zt{h}")
                                        sc = ps.tile([128, 2, QB], F32, tag="ps")
                                        for j in range(2):
                                            kt_i = g * 2 + j
                                            nc.tensor.matmul(
                                                sc[:, j, :],
                                                kT[off:off + 64, hv,
                                                   kt_i * 128:(kt_i + 1) * 128],
                                                qT[off:off + 64, hv,
                                                   qi * QB:(qi + 1) * QB],
                                                start=True, stop=True,
                                            )
                                        at = attp.tile([128, 2, QB], F32R, tag="at")
                                        nc.scalar.activation(
                                            at[:], sc[:],
                                            mybir.ActivationFunctionType.Exp)
                                        if g >= G - 2:
                                            nc.vector.tensor_mul(
                                                at[:], at[:],
                                                masks[:, g - (G - 2), :, :])
                                        if debug_taps and qi == 0 and h == 0 and g == 0:
                                            nc.sync.dma_start(dbg["dbg_at0"][:], at[:])
                                        att_tiles[h, g] = at
                                    if g >= 1:
                                        ap = att_tiles.pop((h, g - 1))
                                        for j in range(2):
                                            kt_i = (g - 1) * 2 + j
                                            nc.tensor.matmul(
                                                zts[h][0:HD + 1, :],
                                                vext[:, kt_i, h, :],
                                                ap[:, j, :],
                                                start=(g - 1 == 0 and j == 0),
                                                stop=(g - 1 == G - 1 and j == 1),
                                            )
                                    if g == G:
                                        zt = zts[h]
                                        scr = smp.tile([128, QB], F32, tag="scr")
                                        bc = smp.tile([128, QB], F32, tag="bc")
                                        if debug_taps and qi == 0 and h == 0:
                                            dzt = smp.tile([128, QB], F32, tag="dzt")
                                            nc.vector.tensor_copy(dzt[:], zt[:])
                                            nc.sync.dma_start(dbg["dbg_zt"][:], dzt[:])
                                        nc.vector.reciprocal(scr[0:1, :], zt[HD:HD + 1, :])
                                        nc.gpsimd.partition_broadcast(
                                            bc[:], scr[0:1, :], channels=128)
                                        nc.vector.tensor_mul(
                                            zTn[off:off + 64, hv, :],
                                            zt[0:HD, :], bc[off:off + 64, :])
                                        if debug_taps and qi == 0 and h == 0:
                                            nc.sync.dma_start(dbg["dbg_scr"][:], scr[:])
                                            nc.sync.dma_start(dbg["dbg_bc"][:], bc[:])
                    if debug_taps and qi == 0:
                        nc.sync.dma_start(dbg["dbg_zTn"][:], zTn[:])

                    with nc.named_scope(f"oproj_q{qi}"):
                        for tch in range(QB // 128):
                            po = ps.tile([128, 2, QB], F32, tag="ps")
                            for et in range(2):
                                for dt_ in range(2):
                                    nc.tensor.matmul(
                                        po[:, et, :],
                                        zTn[:, dt_, tch * 128:(tch + 1) * 128],
                                        wout[:, dt_, et * QB:(et + 1) * QB],
                                        start=(dt_ == 0), stop=(dt_ == 1),
                                    )
                            ot = opsp.tile([128, 2, QB], F32, tag="ot")
                            nc.vector.tensor_copy(ot[:], po[:])
                            row = qi * QB + tch * 128
                            nc.sync.dma_start(
                                out_d[row:row + 128, :],
                                ot.rearrange("p a q -> p (a q)"))

            if reps == 1:
                body(0)
            else:
                with tc.For_i(0, reps, 1) as i:
                    body(i)

    nc.compile()
    return nc


def make_in_maps(X, W_qkv, W_out):
    """Host-side sharding: per-core input dict."""
    X = np.asarray(X, dtype=np.float32)
    W_qkv = np.asarray(W_qkv, dtype=np.float32)
    W_out = np.asarray(W_out, dtype=np.float32)

    kp = np.arange(KT)[:, None]
    qf = np.arange(QB)[None, :]
    masks = np.zeros((KT, 2, 2, QB), dtype=np.float32)
    for pair in range(2):
        for j in range(2):
            r = pair * 256 + j * 128
            masks[:, pair, j, :] = (qf >= kp + r).astype(np.float32)

    in_maps = []
    for c in range(NCORES):
        b, hg = divmod(c, NGROUP)
        rows = slice(hg * DLOC, (hg + 1) * DLOC)
        wq = W_qkv[0 * D:1 * D][rows].T * 0.125   # fold 1/sqrt(hd) into q
        wk = W_qkv[1 * D:2 * D][rows].T
        wv = W_qkv[2 * D:3 * D][rows].T
        in_maps.append({
            "xt": np.ascontiguousarray(X[b].T),
            "wqkv": np.ascontiguousarray(np.concatenate([wq, wk, wv], axis=1)),
            "wout": np.ascontiguousarray(W_out[:, rows].T),
            "masks": masks,
            "ones": np.ones((128, NKT, HPC, 1), dtype=np.float32),
        })
    return in_maps


def combine_outputs(results):
    """Sum the 4 tensor-parallel partials per batch -> [B, T, D]."""
    out = np.zeros((B, T, D), dtype=np.float32)
    for c, r in enumerate(results):
        out[c // NGROUP] += r["out"]
    return out


_cached = {}


def kernel(X, W_qkv, W_out):
    from concourse.bass_utils import run_bass_kernel_spmd

    if "nc" not in _cached:
        _cached["nc"] = build_program(reps=1)
    nc = _cached["nc"]
    in_maps = make_in_maps(X, W_qkv, W_out)
    r = run_bass_kernel_spmd(nc, in_maps, core_ids=list(range(NCORES)))
    return combine_outputs(r.results)


# revision 12
# speedup vs baseline: 1.0740x; 1.0740x over previous
"""Causal self-attention (B=2, T=2048, D=1024, H=16) on 8 TRN2 NeuronCores.

Sharding: data parallel on batch (2 groups of 4 cores) x tensor parallel on
heads (4 heads per core, splitting the qkv / out projections). Each core
computes a partial output ``X[b] -> partial_out`` for its 4 heads; the host
sums the 4 partials per batch. No device collectives.

Per-core math (all matmuls fp32r at full PE rate, N>=256):
  qT/kT  [e,t] = Wqkv_localT.T @ X[b].T          (q pre-scaled by 1/8 on host)
  v      [t,d] = X[b] @ Wv_localT                (natural layout)
  scoresT[k,q] = kT.T @ qT                       (contraction d=64)
  attT   [k,q] = exp(scoresT)   * causal mask    (no max subtraction: |scores|
                                                  is O(1) for this data)
  zT     [d,q] = v_ext.T @ attT                  (v_ext has a ones column ->
                                                  row 64 of zT = softmax denom)
  out    [t,e] = (zT/denom).T @ Wout_localT      (host sums partials over 4
                                                  cores per batch)

Pipelining: the projection work for q-block qi (qk-proj of t-block qi,
v-proj of the k-tiles qi unlocks) is emitted just before attention of qi,
so the PE fills the ACT-bound exp phases with projection matmuls. Heads are
processed in interleaved pairs to deepen the sc->exp->mask->av pipeline.
"""

import sys

sys.path.insert(0, "/opt/trn_rl_repo")

import numpy as np

import concourse.bacc as bacc
import concourse.mybir as mybir
import concourse.tile as tile

B, T, D, H = 2, 2048, 1024, 16
HD = D // H              # 64
NCORES = 8
NGROUP = 4               # cores per batch (tensor-parallel group)
HPC = H // NGROUP        # heads per core = 4
DLOC = HPC * HD          # local model dims per core = 256
QB = 512                 # q-block (matmul moving free dim)
NQB = T // QB            # 4
KT = 128                 # k-tile (psum partitions)
NKT = T // KT            # 16
NDT = D // 128           # 8 d-tiles

F32 = mybir.dt.float32
F32R = mybir.dt.float32r


def build_program(reps=1, debug_taps=False, hoist_loads=False):
    nc = bacc.Bacc("TRN2", target_bir_lowering=False, debug=False,
                   num_devices=NCORES)

    xt_d = nc.declare_dram_parameter("xt", [D, T], F32R, isOutput=False)
    wqkv_d = nc.declare_dram_parameter("wqkv", [D, 3 * DLOC], F32R, isOutput=False)
    wout_d = nc.declare_dram_parameter("wout", [DLOC, D], F32R, isOutput=False)
    mask_d = nc.declare_dram_parameter("masks", [KT, 2, 2, QB], mybir.dt.bfloat16, isOutput=False)
    ones_d = nc.declare_dram_parameter("ones", [128, NKT, HPC, 1], F32R, isOutput=False)
    out_d = nc.declare_dram_parameter("out", [T, D], F32, isOutput=True)
    if debug_taps:
        dbg = {
            "dbg_qT": nc.declare_dram_parameter("dbg_qT", [128, 2, T], F32R, isOutput=True),
            "dbg_kT": nc.declare_dram_parameter("dbg_kT", [128, 2, T], F32R, isOutput=True),
            "dbg_vext": nc.declare_dram_parameter("dbg_vext", [128, NKT, HPC, HD + 1], F32R, isOutput=True),
            "dbg_at0": nc.declare_dram_parameter("dbg_at0", [128, 2, QB], F32R, isOutput=True),
            "dbg_zt": nc.declare_dram_parameter("dbg_zt", [128, QB], F32, isOutput=True),
            "dbg_scr": nc.declare_dram_parameter("dbg_scr", [128, QB], F32, isOutput=True),
            "dbg_bc": nc.declare_dram_parameter("dbg_bc", [128, QB], F32, isOutput=True),
            "dbg_zTn": nc.declare_dram_parameter("dbg_zTn", [128, 2, QB], F32R, isOutput=True),
        }

    with tile.TileContext(nc) as tc:
        with (
            tc.tile_pool(name="cst", bufs=1) as cst,
            tc.tile_pool(name="att", bufs=4) as attp,
            tc.tile_pool(name="sm", bufs=2) as smp,
            tc.tile_pool(name="ops", bufs=2) as opsp,
            tc.tile_pool(name="ps", bufs=3, space="PSUM") as ps,    # [128,2,512]
            tc.tile_pool(name="zps", bufs=2, space="PSUM") as zps,  # [128,512]
        ):
            xt = cst.tile([128, NDT, T], F32R, tag="xt")
            wqkv = cst.tile([128, NDT, 3 * DLOC], F32R, tag="wqkv")
            wout = cst.tile([128, DLOC // 128, D], F32R, tag="wout")
            masks = cst.tile([128, 2, 2, QB], mybir.dt.bfloat16, tag="masks")
            qT = cst.tile([128, 2, T], F32R, tag="qT")
            kT = cst.tile([128, 2, T], F32R, tag="kT")
            vext = cst.tile([128, NKT, HPC, HD + 1], F32R, tag="vext")

            def qk_proj(tb):
                for pair in range(2):  # 0 -> q (e 0:256), 1 -> k
                    pt = ps.tile([128, 2, QB], F32, tag="ps")
                    for j in range(2):
                        ecol = (pair * 2 + j) * 128
                        for dt_ in range(NDT):
                            nc.tensor.matmul(
                                pt[:, j, :],
                                wqkv[:, dt_, ecol:ecol + 128],
                                xt[:, dt_, tb * QB:(tb + 1) * QB],
                                start=(dt_ == 0), stop=(dt_ == NDT - 1),
                            )
                    dst = qT if pair == 0 else kT
                    nc.scalar.copy(dst[:, :, tb * QB:(tb + 1) * QB], pt[:])

            def v_proj(tp):
                pt = ps.tile([128, 2, QB], F32, tag="ps")
                for j in range(2):
                    tch = tp * 2 + j
                    for dt_ in range(NDT):
                        nc.tensor.matmul(
                            pt[:, j, 0:DLOC],
                            xt[:, dt_, tch * 128:(tch + 1) * 128],
                            wqkv[:, dt_, 2 * DLOC:3 * DLOC],
                            start=(dt_ == 0), stop=(dt_ == NDT - 1),
                        )
                    nc.vector.tensor_copy(
                        vext[:, tch, :, 0:HD],
                        pt[:, j, 0:DLOC].rearrange("p (h d) -> p h d", h=HPC),
                    )

            def load_inputs():
                # split big loads across DMA queues; arrival order matches
                # the order the projections consume the data
                wqkv_r = wqkv_d.rearrange("(a p) e -> p a e", p=128)
                xt_r = xt_d.rearrange("(a p) t -> p a t", p=128)
                for dt_ in range(0, NDT, 2):   # q/k weight columns first
                    nc.sync.dma_start(wqkv[:, dt_:dt_ + 2, 0:2 * DLOC],
                                      wqkv_r[:, dt_:dt_ + 2, 0:2 * DLOC])
                for half in range(2):          # first t-block in 2 pieces
                    sl = slice(half * 256, (half + 1) * 256)
                    nc.sync.dma_start(xt[:, :, sl], xt_r[:, :, sl])
                for dt_ in range(0, NDT, 2):   # v weight columns
                    nc.sync.dma_start(wqkv[:, dt_:dt_ + 2, 2 * DLOC:3 * DLOC],
                                      wqkv_r[:, dt_:dt_ + 2, 2 * DLOC:3 * DLOC])
                for tchunk in range(1, 4):
                    sl = slice(tchunk * QB, (tchunk + 1) * QB)
                    nc.sync.dma_start(xt[:, :, sl], xt_r[:, :, sl])
                nc.sync.dma_start(wout[:], wout_d.rearrange("(a p) e -> p a e", p=128))
                nc.sync.dma_start(masks[:], mask_d[:])
                nc.sync.dma_start(vext[:, :, :, HD:HD + 1], ones_d[:])

            def body(_i):
                if not hoist_loads:
                    load_inputs()
                zTn_prev = {}
                for qi in range(NQB):
                    with nc.named_scope(f"proj_q{qi}"):
                        qk_proj(qi)
                        v_proj(2 * qi)
                        v_proj(2 * qi + 1)
                    if debug_taps and qi == NQB - 1:
                        nc.sync.dma_start(dbg["dbg_qT"][:], qT[:])
                        nc.sync.dma_start(dbg["dbg_kT"][:], kT[:])
                        nc.sync.dma_start(dbg["dbg_vext"][:], vext[:])

                    zTn = smp.tile([128, 2, QB], F32R, tag="zTn")
                    zTn_prev[qi] = zTn
                    G = 2 * (qi + 1)          # k-groups of 2 k-tiles
                    for p in range(2):        # head pairs (0,1) then (2,3)
                        zts = {}
                        att_tiles = {}
                        with nc.named_scope(f"att_q{qi}_p{p}"):
                            for g in range(G + 1):
                                for h in (2 * p, 2 * p + 1):
                                    off, hv = (h % 2) * 64, h // 2
                                    if g < G:
                                        if g == 0:
                                            zts[h] = zps.tile([128, QB], F32, tag="zt", name=f"zt{h}")
                                        sc = ps.tile([128, 2, QB], F32, tag="ps")
                                        for j in range(2):
                                            kt_i = g * 2 + j
                                            nc.tensor.matmul(
                                                sc[:, j, :],
                                                kT[off:off + 64, hv,
                                                   kt_i * 128:(kt_i + 1) * 128],
                                                qT[off:off + 64, hv,
                                                   qi * QB:(qi + 1) * QB],
                                                start=True, stop=True,
                                            )
                                        at = attp.tile([128, 2, QB], F32R, tag="at")
                                        nc.scalar.activation(
                                            at[:], sc[:],
                                            mybir.ActivationFunctionType.Exp)
                                        if g >= G - 2:
                                            nc.vector.tensor_mul(
                                                at[:], at[:],
                                                masks[:, g - (G - 2), :, :])
                                        if debug_taps and qi == 0 and h == 0 and g == 0:
                                            nc.sync.dma_start(dbg["dbg_at0"][:], at[:])
                                        att_tiles[h, g] = at
                                    if g >= 1:
                                        ap = att_tiles.pop((h, g - 1))
                                        for j in range(2):
                                            kt_i = (g - 1) * 2 + j
                                            nc.tensor.matmul(
                                                zts[h][0:HD + 1, :],
                                                vext[:, kt_i, h, :],
                                                ap[:, j, :],
                                                start=(g - 1 == 0 and j == 0),
                                                stop=(g - 1 == G - 1 and j == 1),
                                            )
                                    if g == G:
                                        zt = zts[h]
                                        scr = smp.tile([128, QB], F32, tag="scr")
                                        bc = smp.tile([128, QB], F32, tag="bc")
                                        if debug_taps and qi == 0 and h == 0:
                                            dzt = smp.tile([128, QB], F32, tag="dzt")
                                            nc.vector.tensor_copy(dzt[:], zt[:])
                                            nc.sync.dma_start(dbg["dbg_zt"][:], dzt[:])
                                        nc.vector.reciprocal(scr[0:1, :], zt[HD:HD + 1, :])
                                        nc.gpsimd.partition_broadcast(
                                            bc[:], scr[0:1, :], channels=128)
                                        nc.vector.tensor_mul(
                                            zTn[off:off + 64, hv, :],
                                            zt[0:HD, :], bc[off:off + 64, :])
                                        if debug_taps and qi == 0 and h == 0:
                                            nc.sync.dma_start(dbg["dbg_scr"][:], scr[:])
                                            nc.sync.dma_start(dbg["dbg_bc"][:], bc[:])
                    if debug_taps and qi == 0:
                        nc.sync.dma_start(dbg["dbg_zTn"][:], zTn[:])

                    # output projection, delayed one q-block so the
                    # zTn-completion wait hides under the next attention
                    def oproj(qo):
                        zo = zTn_prev.pop(qo)
                        with nc.named_scope(f"oproj_q{qo}"):
                            for tch in range(QB // 128):
                                po = ps.tile([128, 2, QB], F32, tag="ps")
                                for et in range(2):
                                    for dt_ in range(2):
                                        nc.tensor.matmul(
                                            po[:, et, :],
                                            zo[:, dt_, tch * 128:(tch + 1) * 128],
                                            wout[:, dt_, et * QB:(et + 1) * QB],
                                            start=(dt_ == 0), stop=(dt_ == 1),
                                        )
                                ot = opsp.tile([128, 2, QB], F32, tag="ot")
                                nc.vector.tensor_copy(ot[:], po[:])
                                row = qo * QB + tch * 128
                                nc.sync.dma_start(
                                    out_d[row:row + 128, :],
                                    ot.rearrange("p a q -> p (a q)"))

                    if qi >= 1:
                        oproj(qi - 1)
                    if qi == NQB - 1:
                        oproj(qi)

            if hoist_loads:
                load_inputs()
            if reps == 1:
                body(0)
            else:
                with tc.For_i(0, reps, 1) as i:
                    body(i)

    nc.compile()
    return nc


def make_in_maps(X, W_qkv, W_out):
    """Host-side sharding: per-core input dict."""
    X = np.asarray(X, dtype=np.float32)
    W_qkv = np.asarray(W_qkv, dtype=np.float32)
    W_out = np.asarray(W_out, dtype=np.float32)

    kp = np.arange(KT)[:, None]
    qf = np.arange(QB)[None, :]
    import ml_dtypes
    masks = np.zeros((KT, 2, 2, QB), dtype=ml_dtypes.bfloat16)
    for pair in range(2):
        for j in range(2):
            r = pair * 256 + j * 128
            masks[:, pair, j, :] = (qf >= kp + r)

    in_maps = []
    for c in range(NCORES):
        b, hg = divmod(c, NGROUP)
        rows = slice(hg * DLOC, (hg + 1) * DLOC)
        wq = W_qkv[0 * D:1 * D][rows].T * 0.125   # fold 1/sqrt(hd) into q
        wk = W_qkv[1 * D:2 * D][rows].T
        wv = W_qkv[2 * D:3 * D][rows].T
        in_maps.append({
            "xt": np.ascontiguousarray(X[b].T),
            "wqkv": np.ascontiguousarray(np.concatenate([wq, wk, wv], axis=1)),
            "wout": np.ascontiguousarray(W_out[:, rows].T),
            "masks": masks,
            "ones": np.ones((128, NKT, HPC, 1), dtype=np.float32),
        })
    return in_maps


def combine_outputs(results):
    """Sum the 4 tensor-parallel partials per batch -> [B, T, D]."""
    out = np.zeros((B, T, D), dtype=np.float32)
    for c, r in enumerate(results):
        out[c // NGROUP] += r["out"]
    return out


_cached = {}


def kernel(X, W_qkv, W_out):
    from concourse.bass_utils import run_bass_kernel_spmd

    if "nc" not in _cached:
        _cached["nc"] = build_program(reps=1)
    nc = _cached["nc"]
    in_maps = make_in_maps(X, W_qkv, W_out)
    r = run_bass_kernel_spmd(nc, in_maps, core_ids=list(range(NCORES)))
    return combine_outputs(r.results)
